# revision 1
# baseline (speedup 1.0000x reference)
"""Multi-head attention (B=2, S=2048, D=1024, H=16, Dk=64) on 8 TRN2 cores.

Sharding: batch-split x head-TP.  Core c handles batch c//4 and heads
hs*4..hs*4+3 where hs = c%4 (256 projection dims = 2 "ob" blocks of 128).
Each core:
  1. projects kT/vT/qT = (W_slice.T @ x.T) for its 4 heads   [2x[128, 2048]]
  2. transposes vT into per-(ob,h) [j, d] blocks with an appended
     ones-column (so P@V_aug also yields the softmax row-sums)
  3. pipelined attention per (ob, half): scoresT -> exp (FD=1024 ACT)
     -> PV accumulate [65, 1024] PSUM; the 1/rowsum PE-broadcast lands in
     partitions 64:128 of the same PSUM banks.
  4. partialT = Wo_slice.T @ oT  (K=256 accumulated over both obs)
Host sums 4 partials per batch, adds bo, transposes back.

All matmuls fp16 operands with fp32 PSUM accumulation.
"""

import numpy as np

D = 1024
S = 2048  # tokens per batch (= per core)
B = 2
N_CORES = 8

_CACHE = {}


def _build_nc(mm_dtype="float16"):
    import concourse.bacc as bacc
    import concourse.mybir as mybir
    import concourse.tile as tile

    dt = mybir.dt
    f32 = dt.float32
    mmdt = getattr(dt, mm_dtype)
    AF = mybir.ActivationFunctionType

    nc = bacc.Bacc("TRN2", target_bir_lowering=False, debug=False)

    xq = nc.dram_tensor("xq", [D, S], mmdt, kind="ExternalInput").ap()
    xk = nc.dram_tensor("xk", [D, S], mmdt, kind="ExternalInput").ap()
    xv = nc.dram_tensor("xv", [D, S], mmdt, kind="ExternalInput").ap()
    wq = nc.dram_tensor("wq", [128, 2048], mmdt, kind="ExternalInput").ap()
    wk = nc.dram_tensor("wk", [128, 2048], mmdt, kind="ExternalInput").ap()
    wv = nc.dram_tensor("wv", [128, 2048], mmdt, kind="ExternalInput").ap()
    wo = nc.dram_tensor("wo", [128, 2048], mmdt, kind="ExternalInput").ap()
    bias6 = nc.dram_tensor("bias6", [128, 6], f32, kind="ExternalInput").ap()
    c_ident = nc.dram_tensor("c_ident", [128, 64], mmdt, kind="ExternalInput").ap()
    c_ones64 = nc.dram_tensor("c_ones64", [1, 64], mmdt, kind="ExternalInput").ap()
    pout = nc.dram_tensor("pout", [D, S], mmdt, kind="ExternalOutput").ap()

    with tile.TileContext(nc) as tc:
        from contextlib import ExitStack

        with ExitStack() as stk:
            const = stk.enter_context(tc.tile_pool(name="const", bufs=1))
            wpool = stk.enter_context(tc.tile_pool(name="w", bufs=1))
            big = stk.enter_context(tc.tile_pool(name="big", bufs=1))
            xpool = stk.enter_context(tc.tile_pool(name="xt", bufs=6))
            ptp = stk.enter_context(tc.tile_pool(name="pt", bufs=4))
            rsp = stk.enter_context(tc.tile_pool(name="rs", bufs=2))
            stp = stk.enter_context(tc.tile_pool(name="st", bufs=4))

            # ---- constants ----
            ident = const.tile([128, 64], mmdt)
            nc.sync.dma_start(out=ident, in_=c_ident)
            ones64 = const.tile([1, 64], mmdt)
            nc.sync.dma_start(out=ones64, in_=c_ones64)
            bias_sb = const.tile([128, 6], f32)
            nc.sync.dma_start(out=bias_sb, in_=bias6)

            # ---- weights ----
            wq_sb = wpool.tile([128, 2048], mmdt)
            wk_sb = wpool.tile([128, 2048], mmdt)
            wv_sb = wpool.tile([128, 2048], mmdt)
            wo_sb = wpool.tile([128, 2048], mmdt)
            nc.sync.dma_start(out=wk_sb, in_=wk)
            nc.sync.dma_start(out=wv_sb, in_=wv)
            nc.sync.dma_start(out=wq_sb, in_=wq)
            nc.sync.dma_start(out=wo_sb, in_=wo)

            # ---- persistent activations ----
            qT2 = big.tile([128, 4096], mmdt)  # [dh within ob, ob*2048 + tok]
            kT2 = big.tile([128, 4096], mmdt)
            vT2 = big.tile([128, 4096], mmdt)
            oT2 = big.tile([128, 4096], mmdt)
            v_sb = big.tile([128, 4 * 16 * 65], mmdt)  # [j, (ob,h)*jt*(64+1)]
            nc.vector.memset(v_sb, 1.0)
            v_r = v_sb.rearrange("p (t c) -> p t c", c=65)

            def emit_proj(x_dram, w_sb, dst, bias_col0, pnm):
                """dst[:, ob*2048 + tok] = W.T @ x + b for both ob blocks."""
                with tc.tile_pool(name=f"pp{pnm}", bufs=8, space="PSUM") as pp:
                    acc = [
                        pp.tile([128, 512], f32, tag="pp", name=f"acc{pnm}_{a}")
                        for a in range(8)
                    ]
                    for kk in range(8):
                        x_t = xpool.tile([128, 2048], mmdt, tag="xt", name=f"x{pnm}{kk}")
                        nc.sync.dma_start(
                            out=x_t, in_=x_dram[kk * 128 : (kk + 1) * 128, :]
                        )
                        for ob in range(2):
                            for n in range(4):
                                nc.tensor.matmul(
                                    acc[ob * 4 + n],
                                    lhsT=w_sb[:, (kk * 2 + ob) * 128 : (kk * 2 + ob + 1) * 128],
                                    rhs=x_t[:, n * 512 : (n + 1) * 512],
                                    start=(kk == 0),
                                    stop=(kk == 7),
                                )
                    for ob in range(2):
                        for n in range(4):
                            nc.vector.tensor_scalar_add(
                                dst[:, ob * 2048 + n * 512 : ob * 2048 + (n + 1) * 512],
                                acc[ob * 4 + n],
                                bias_sb[:, bias_col0 + ob : bias_col0 + ob + 1],
                            )

            def emit_transp():
                """vT2 -> v_sb [j, d] blocks for all 4 head-slots."""
                with tc.tile_pool(name="tp", bufs=3, space="PSUM") as tpp:
                    for ob in range(2):
                        for h in range(2):
                            bh = ob * 2 + h
                            for g in range(4):
                                tp = tpp.tile(
                                    [128, 4 * 64], mmdt, tag="tp", name=f"tp{bh}_{g}"
                                )
                                for u in range(4):
                                    jb = g * 4 + u
                                    nc.tensor.transpose(
                                        tp[:, u * 64 : (u + 1) * 64],
                                        vT2[
                                            h * 64 : (h + 1) * 64,
                                            ob * 2048 + jb * 128 : ob * 2048 + (jb + 1) * 128,
                                        ],
                                        ident[h * 64 : (h + 1) * 64, :],
                                    )
                                tp_r = tp.rearrange("p (t c) -> p t c", c=64)
                                nc.scalar.copy(
                                    v_r[:, bh * 16 + g * 4 : bh * 16 + g * 4 + 4, 0:64],
                                    tp_r,
                                )

            # =========== emission schedule ===========
            emit_proj(xk, wk_sb, kT2, 2, "k")
            emit_proj(xv, wv_sb, vT2, 4, "v")
            emit_transp()
            emit_proj(xq, wq_sb, qT2, 0, "q")

            # ---- attention: pipelined over (ob, half, jt) ----
            def emit_finalize(o_ps, i0, tag):
                for h in range(2):
                    rinv = rsp.tile([1, 1024], mmdt, tag="ri", name=f"ri{tag}_{h}")
                    with nc.allow_low_precision(reason="fp16 rinv is plenty"):
                        nc.vector.reciprocal(rinv, o_ps[h][64:65, :])
                    for c in range(2):
                        nc.tensor.matmul(
                            o_ps[h][64:128, c * 512 : (c + 1) * 512],
                            lhsT=ones64,
                            rhs=rinv[:, c * 512 : (c + 1) * 512],
                            start=True,
                            stop=True,
                        )
                    Rs = rsp.tile([64, 1024], f32, tag="rs", name=f"Rs{tag}_{h}")
                    nc.vector.tensor_copy(Rs, o_ps[h][64:128, :])
                    nc.vector.tensor_mul(
                        oT2[h * 64 : (h + 1) * 64, i0 : i0 + 1024],
                        o_ps[h][0:64, :],
                        Rs,
                    )

            with (
                tc.tile_pool(name="scp", bufs=2, space="PSUM") as scp,
                tc.tile_pool(name="opp", bufs=2, space="PSUM") as opp,
            ):
                pending = None
                for ob in range(2):
                    for half in range(2):
                        i0 = ob * 2048 + half * 1024
                        o_ps = [
                            opp.tile([128, 1024], f32, tag="ops", name=f"o{ob}_{half}_{h}")
                            for h in range(2)
                        ]
                        pt_prev = [None, None]
                        for jt in range(17):
                            for h in range(2):
                                if jt < 16:
                                    sc = scp.tile(
                                        [128, 1024], f32, tag="sc",
                                        name=f"s{ob}_{half}_{jt}_{h}",
                                    )
                                    for c in range(2):
                                        nc.tensor.matmul(
                                            sc[:, c * 512 : (c + 1) * 512],
                                            lhsT=kT2[
                                                h * 64 : (h + 1) * 64,
                                                ob * 2048 + jt * 128 : ob * 2048 + (jt + 1) * 128,
                                            ],
                                            rhs=qT2[
                                                h * 64 : (h + 1) * 64,
                                                i0 + c * 512 : i0 + (c + 1) * 512,
                                            ],
                                            start=True,
                                            stop=True,
                                        )
                                    pt = ptp.tile(
                                        [128, 1024], mmdt, tag="pt",
                                        name=f"p{ob}_{half}_{jt}_{h}",
                                    )
                                    nc.scalar.activation(pt, sc, AF.Exp, scale=0.125)
                                if jt > 0:
                                    jp = jt - 1
                                    bh = ob * 2 + h
                                    for c in range(2):
                                        nc.tensor.matmul(
                                            o_ps[h][0:65, c * 512 : (c + 1) * 512],
                                            lhsT=v_sb[
                                                :, (bh * 16 + jp) * 65 : (bh * 16 + jp + 1) * 65
                                            ],
                                            rhs=pt_prev[h][:, c * 512 : (c + 1) * 512],
                                            start=(jp == 0),
                                            stop=(jp == 15),
                                        )
                                if jt < 16:
                                    pt_prev[h] = pt
                            # deferred finalize of the previous (ob, half):
                            # emitted after this half's prologue is in flight
                            if jt == 1 and pending is not None:
                                emit_finalize(*pending)
                                pending = None
                        pending = (o_ps, i0, f"{ob}_{half}")
                emit_finalize(*pending)

            # ---- out-projection: partial = Wo_slice.T @ oT (K=256) ----
            with tc.tile_pool(name="opj", bufs=4, space="PSUM") as pj:
                for dtb in range(8):
                    ops = [
                        pj.tile([128, 512], f32, tag="pj", name=f"pj{dtb}_{c}")
                        for c in range(4)
                    ]
                    for ob in range(2):
                        for c in range(4):
                            nc.tensor.matmul(
                                ops[c],
                                lhsT=wo_sb[:, (ob * 8 + dtb) * 128 : (ob * 8 + dtb + 1) * 128],
                                rhs=oT2[:, ob * 2048 + c * 512 : ob * 2048 + (c + 1) * 512],
                                start=(ob == 0),
                                stop=(ob == 1),
                            )
                    for c in range(4):
                        st = stp.tile([128, 512], mmdt, tag="st", name=f"st{dtb}_{c}")
                        eng = nc.vector.tensor_copy if c % 2 else nc.scalar.copy
                        eng(st, ops[c])
                        nc.sync.dma_start(
                            out=pout[
                                dtb * 128 : (dtb + 1) * 128, c * 512 : (c + 1) * 512
                            ],
                            in_=st,
                        )

    nc.compile()
    return nc


MM_DTYPE = "float16"


def _get_nc():
    key = ("nc", MM_DTYPE)
    if key not in _CACHE:
        _CACHE[key] = _build_nc(MM_DTYPE)
    return _CACHE[key]


def _ensure_ntff_hook():
    """Register the NTFF profile hook module if the image lacks it."""
    import sys
    import types

    if "antenv.axon_hooks" in sys.modules:
        return
    try:
        from trn_agent_boot.trn_boot import _ntff_profile_via_ctypes
    except Exception:
        return
    hook = None
    try:
        hook = _ntff_profile_via_ctypes("/opt/axon/libaxon_pjrt.so")
    except Exception:
        hook = None
    mod = types.ModuleType("antenv.axon_hooks")
    mod._hook = hook
    mod.get_axon_ntff_profile_hook = lambda: mod._hook
    mod.set_axon_ntff_profile_hook = lambda h: setattr(mod, "_hook", h)
    sys.modules["antenv.axon_hooks"] = mod


def _make_in_maps(inputs, ext_dt):
    query = np.asarray(inputs["query"], np.float32)
    key = np.asarray(inputs["key"], np.float32)
    value = np.asarray(inputs["value"], np.float32)
    Wq = np.asarray(inputs["Wq"], np.float32)
    Wk = np.asarray(inputs["Wk"], np.float32)
    Wv = np.asarray(inputs["Wv"], np.float32)
    Wo = np.asarray(inputs["Wo"], np.float32)
    bq = np.asarray(inputs["bq"], np.float32)
    bk = np.asarray(inputs["bk"], np.float32)
    bv = np.asarray(inputs["bv"], np.float32)

    # per-batch transposed inputs [D, S]
    xT = {}
    for b in range(B):
        xT[("q", b)] = np.ascontiguousarray(query[b].T.astype(ext_dt))
        xT[("k", b)] = np.ascontiguousarray(key[b].T.astype(ext_dt))
        xT[("v", b)] = np.ascontiguousarray(value[b].T.astype(ext_dt))

    ident_np = np.zeros((128, 64), np.float32)
    ident_np[np.arange(64), np.arange(64)] = 1.0
    ident_np[64 + np.arange(64), np.arange(64)] = 1.0
    consts = {
        "c_ident": np.ascontiguousarray(ident_np.astype(ext_dt)),
        "c_ones64": np.ones((1, 64), ext_dt),
    }

    def pack_w(Wc):  # [1024, 256] -> [128, 2048] as (kk, ob) tiles
        return np.ascontiguousarray(
            Wc.reshape(8, 128, 2, 128).transpose(1, 0, 2, 3).reshape(128, 2048).astype(ext_dt)
        )

    def pack_wo(Wc):  # [256, 1024] -> [128, 2048] as (ob, dt) tiles
        return np.ascontiguousarray(
            Wc.reshape(2, 128, 8, 128).transpose(1, 0, 2, 3).reshape(128, 2048).astype(ext_dt)
        )

    in_maps = []
    for c in range(N_CORES):
        b, hs = divmod(c, 4)
        sl = slice(hs * 256, (hs + 1) * 256)
        bias6 = np.zeros((128, 6), np.float32)
        bias6[:, 0] = bq[sl][0:128]
        bias6[:, 1] = bq[sl][128:256]
        bias6[:, 2] = bk[sl][0:128]
        bias6[:, 3] = bk[sl][128:256]
        bias6[:, 4] = bv[sl][0:128]
        bias6[:, 5] = bv[sl][128:256]
        in_maps.append(
            {
                **consts,
                "xq": xT[("q", b)],
                "xk": xT[("k", b)],
                "xv": xT[("v", b)],
                "wq": pack_w(Wq[:, sl]),
                "wk": pack_w(Wk[:, sl]),
                "wv": pack_w(Wv[:, sl]),
                "wo": pack_wo(Wo[sl, :]),
                "bias6": np.ascontiguousarray(bias6),
            }
        )
    return in_maps


def _gather(results, bo):
    outT = np.zeros((B, D, S), np.float64)
    for c in range(N_CORES):
        outT[c // 4] += np.asarray(results[c]["pout"], np.float64)
    out = outT.transpose(0, 2, 1) + bo.astype(np.float64)
    return out.astype(np.float32)


def _run(inputs, trace=False):
    from concourse import bass_utils

    if trace:
        _ensure_ntff_hook()

    nc = _get_nc()
    if MM_DTYPE == "bfloat16":
        import ml_dtypes

        ext_dt = ml_dtypes.bfloat16
    elif MM_DTYPE == "float16":
        ext_dt = np.float16
    else:
        ext_dt = np.float32

    in_maps = _make_in_maps(inputs, ext_dt)
    res = bass_utils.run_bass_kernel_spmd(
        nc, in_maps, core_ids=list(range(N_CORES)), trace=trace
    )
    bo = np.asarray(inputs["bo"], np.float32)
    out = _gather(res.results, bo)
    return out.reshape(B, S, D), res


def kernel(**inputs):
    out, _ = _run(inputs, trace=False)
    return out



# revision 25
# speedup vs baseline: 1.0899x; 1.0899x over previous
"""Multi-head attention (B=2, S=2048, D=1024, H=16, Dk=64) on 8 TRN2 cores.

Sharding: batch-split x head-TP.  Core c handles batch c//4 and heads
hs*4..hs*4+3 where hs = c%4 (256 projection dims = 2 "ob" blocks of 128).
Each core:
  1. projects kT/qT/vT = (W_slice.T @ x.T) for its 4 heads   [2x[128, 2048]]
  2. transposes vT into per-(ob,h) fp8 [j, 2, d] DoubleRow blocks with an
     appended ones-column (so P@V_aug also yields the softmax row-sums)
  3. pipelined attention per (ob, half): scoresT (h0/h1 emitted as adjacent
     row-group pairs for PE tile concurrency) -> exp (FD=1024 ACT, fp8 out)
     -> PV fp8 DoubleRow accumulate [65, 1024] PSUM; finalize uses
     reciprocal_approx_fast + fp32 ones-broadcast matmul.
  4. partialT = Wo_slice.T @ oT  (K=256 accumulated over both obs)
Host sums 4 partials per batch, adds bo, transposes back.

QKV/out-proj matmuls fp16; PV fp8e4m3 (noise washes out through softmax
averaging); fp32 PSUM accumulation everywhere.
"""

import numpy as np

D = 1024
S = 2048  # tokens per batch (= per core)
B = 2
N_CORES = 8

PV_FP8 = False  # fp8 DoubleRow PV is ~3.4% rms (pt+v quantization doesn't
# wash out through softmax averaging) — too lossy for the 2e-2 gate
DEBUG_DUMPS = False  # extra DRAM outputs for bisection

_CACHE = {}


def _build_nc(mm_dtype="float16"):
    import concourse.bacc as bacc
    import concourse.mybir as mybir
    import concourse.tile as tile

    dt = mybir.dt
    f32 = dt.float32
    f8 = dt.float8e4
    mmdt = getattr(dt, mm_dtype)
    pvdt = f8 if PV_FP8 else mmdt
    AF = mybir.ActivationFunctionType
    DR = mybir.MatmulPerfMode.DoubleRow

    f32r = dt.float32r
    EXP_BIAS = -2.5 if PV_FP8 else 0.0  # keep exp() under fp8e4m3 max 448;
    # cancels exactly in the softmax normalization

    nc = bacc.Bacc("TRN2", target_bir_lowering=False, debug=False)

    xq = nc.dram_tensor("xq", [D, S], mmdt, kind="ExternalInput").ap()
    xk = nc.dram_tensor("xk", [D, S], mmdt, kind="ExternalInput").ap()
    xv = nc.dram_tensor("xv", [D, S], mmdt, kind="ExternalInput").ap()
    wq = nc.dram_tensor("wq", [128, 2048], mmdt, kind="ExternalInput").ap()
    wk = nc.dram_tensor("wk", [128, 2048], mmdt, kind="ExternalInput").ap()
    wv = nc.dram_tensor("wv", [128, 2048], mmdt, kind="ExternalInput").ap()
    wo = nc.dram_tensor("wo", [128, 2048], mmdt, kind="ExternalInput").ap()
    bias6 = nc.dram_tensor("bias6", [128, 6], f32, kind="ExternalInput").ap()
    c_ident = nc.dram_tensor("c_ident", [128, 64], mmdt, kind="ExternalInput").ap()
    c_ones64f = nc.dram_tensor("c_ones64f", [1, 64], f32, kind="ExternalInput").ap()
    pout = nc.dram_tensor("pout", [D, S], mmdt, kind="ExternalOutput").ap()
    if DEBUG_DUMPS:
        d_vsb = nc.dram_tensor("d_vsb", [128, 4 * 8 * 2 * 80], f32, kind="ExternalOutput").ap()
        d_ot2 = nc.dram_tensor("d_ot2", [128, 4096], f32, kind="ExternalOutput").ap()
        d_rs = nc.dram_tensor("d_rs", [8, 1024], f32, kind="ExternalOutput").ap()
        d_pt = nc.dram_tensor("d_pt", [128, 2048], f32, kind="ExternalOutput").ap()

    # v_sb layout: [j=128, (bh, pair, k, c)] fp8, c padded 65 -> 80 so the
    # DoubleRow weight AP's pair-step (80B) stays 16B-aligned.
    VC = 80 if PV_FP8 else 65

    with tile.TileContext(nc) as tc:
        from contextlib import ExitStack

        with ExitStack() as stk:
            const = stk.enter_context(tc.tile_pool(name="const", bufs=1))
            wpool = stk.enter_context(tc.tile_pool(name="w", bufs=1))
            big = stk.enter_context(tc.tile_pool(name="big", bufs=1))
            xpool = stk.enter_context(tc.tile_pool(name="xt", bufs=8))
            ptp = stk.enter_context(tc.tile_pool(name="pt", bufs=4))
            rsp = stk.enter_context(tc.tile_pool(name="rs", bufs=2))
            stp = stk.enter_context(tc.tile_pool(name="st", bufs=4))

            # ---- constants ----
            ident = const.tile([128, 64], mmdt)
            nc.sync.dma_start(out=ident, in_=c_ident)
            ones64f = const.tile([1, 64], f32)
            nc.sync.dma_start(out=ones64f, in_=c_ones64f)
            bias_sb = const.tile([128, 6], f32)
            nc.sync.dma_start(out=bias_sb, in_=bias6)
            # preload the exp table set while projections run
            warm = const.tile([128, 1], f32)
            nc.scalar.activation(warm, bias_sb[:, 0:1], AF.Exp, scale=0.0)
            expb = const.tile([128, 1], f32)
            nc.vector.memset(expb, EXP_BIAS)
            ones16 = const.tile([1, 64], mmdt)
            nc.vector.memset(ones16, 1.0)

            def dma_split(dst, src, nq=4):
                """Split a [128, N] HBM->SBUF load across nq DMA queues
                (per-queue BW is ~23 GB/s; a 512KB tile on one queue = 23us)."""
                step = 128 // nq
                for q in range(nq):
                    nc.sync.dma_start(
                        out=dst[q * step : (q + 1) * step, :],
                        in_=src[q * step : (q + 1) * step, :],
                    )

            # ---- weights (wo deferred until the out-projection) ----
            wq_sb = wpool.tile([128, 2048], mmdt)
            wk_sb = wpool.tile([128, 2048], mmdt)
            wv_sb = wpool.tile([128, 2048], mmdt)
            wo_sb = wpool.tile([128, 2048], mmdt)
            dma_split(wk_sb, wk)
            dma_split(wq_sb, wq)
            dma_split(wv_sb, wv)

            # ---- persistent activations ----
            qT2 = big.tile([128, 4096], mmdt)  # [dh within ob, ob*2048 + tok]
            kT2 = big.tile([128, 4096], mmdt)
            vT2 = big.tile([128, 4096], mmdt)
            oT2 = big.tile([128, 4096], mmdt)
            if PV_FP8:
                v_sb = big.tile([128, 4 * 8 * 2 * VC], pvdt)
                nc.vector.memset(v_sb, 1.0)
                # [j, bh, pair, k, c]
                v_r = v_sb.rearrange("p (b t k c) -> p b t k c", b=4, t=8, k=2)
            else:
                v_sb = big.tile([128, 4 * 16 * VC], pvdt)
                nc.vector.memset(v_sb, 1.0)
                v_r = v_sb.rearrange("p (t c) -> p t c", c=VC)

            def emit_proj(x_dram, w_sb, dst, bias_col0, pnm, pp):
                """dst[:, ob*2048 + tok] = W.T @ x + b for both ob blocks.
                Per-ob acc quads rotate through the shared 8-slot pool so
                consecutive projections pipeline without a pool-release stall."""
                x_ts = []
                for kk in range(8):
                    x_t = xpool.tile([128, 2048], mmdt, tag="xt", name=f"x{pnm}{kk}")
                    dma_split(x_t, x_dram[kk * 128 : (kk + 1) * 128, :])
                    x_ts.append(x_t)
                for ob in range(2):
                    acc = [
                        pp.tile([128, 512], f32, tag="pp", name=f"acc{pnm}{ob}_{n}")
                        for n in range(4)
                    ]
                    for kk in range(8):
                        for n in range(4):
                            nc.tensor.matmul(
                                acc[n],
                                lhsT=w_sb[:, (kk * 2 + ob) * 128 : (kk * 2 + ob + 1) * 128],
                                rhs=x_ts[kk][:, n * 512 : (n + 1) * 512],
                                start=(kk == 0),
                                stop=(kk == 7),
                            )
                    for n in range(4):
                        dstv = dst[:, ob * 2048 + n * 512 : ob * 2048 + (n + 1) * 512]
                        bv = bias_sb[:, bias_col0 + ob : bias_col0 + ob + 1]
                        if n < 2:
                            nc.vector.tensor_scalar_add(dstv, acc[n], bv)
                        else:
                            nc.scalar.activation(dstv, acc[n], AF.Identity, bias=bv)

            def emit_transp():
                """vT2 -> v_sb [j, (k,) d] blocks for all 4 head-slots."""
                with tc.tile_pool(name="tp", bufs=3, space="PSUM") as tpp:
                    for ob in range(2):
                        for h in range(2):
                            bh = ob * 2 + h
                            for g in range(4):
                                tp = tpp.tile(
                                    [128, 4 * 64], mmdt, tag="tp", name=f"tp{bh}_{g}"
                                )
                                for u in range(4):
                                    jb = g * 4 + u
                                    nc.tensor.transpose(
                                        tp[:, u * 64 : (u + 1) * 64],
                                        vT2[
                                            h * 64 : (h + 1) * 64,
                                            ob * 2048 + jb * 128 : ob * 2048 + (jb + 1) * 128,
                                        ],
                                        ident[h * 64 : (h + 1) * 64, :],
                                    )
                                if PV_FP8:
                                    tp_r = tp.rearrange(
                                        "p (t k c) -> p t k c", t=2, k=2, c=64
                                    )
                                    nc.scalar.copy(
                                        v_r[:, bh, 2 * g : 2 * g + 2, 0:2, 0:64],
                                        tp_r,
                                    )
                                else:
                                    tp_r = tp.rearrange("p (t c) -> p t c", c=64)
                                    nc.scalar.copy(
                                        v_r[:, bh * 16 + g * 4 : bh * 16 + g * 4 + 4, 0:64],
                                        tp_r,
                                    )

            # =========== emission schedule ===========
            with tc.tile_pool(name="pp", bufs=8, space="PSUM") as pp:
                emit_proj(xk, wk_sb, kT2, 2, "k", pp)
                emit_proj(xq, wq_sb, qT2, 0, "q", pp)
                emit_proj(xv, wv_sb, vT2, 4, "v", pp)
            emit_transp()
            if DEBUG_DUMPS and PV_FP8:
                vdump = big.tile([128, 4 * 8 * 2 * 80], f32, name="vdump")
                nc.vector.tensor_copy(vdump, v_sb)
                nc.sync.dma_start(out=d_vsb, in_=vdump)

            # ---- attention: pipelined over (ob, half, jt-pair) ----
            def emit_fin_recip(o_ps, tag, dbg_idx=None):
                """Part 1: 1/rowsum via fast approx, straight from PSUM."""
                rinvs = []
                for h in range(2):
                    if DEBUG_DUMPS and dbg_idx is not None:
                        rd = rsp.tile([1, 1024], f32, tag="rd", name=f"rd{tag}_{h}")
                        nc.vector.tensor_copy(rd, o_ps[h][64:65, :])
                        nc.sync.dma_start(
                            out=d_rs[dbg_idx * 2 + h : dbg_idx * 2 + h + 1, :], in_=rd
                        )
                    # custom-DVE ops ignore the input base partition: stage the
                    # rowsum row at partition 0 first
                    rsum = rsp.tile([1, 1024], f32, tag="rsm", name=f"rm{tag}_{h}")
                    nc.vector.tensor_copy(rsum, o_ps[h][64 : 65, :])
                    rinv = rsp.tile([1, 1024], f32, tag="ri", name=f"ri{tag}_{h}")
                    nc.vector.reciprocal_approx_fast(rinv, rsum)
                    r16 = rsp.tile([1, 1024], mmdt, tag="ri16", name=f"rj{tag}_{h}")
                    with nc.allow_low_precision(reason="fp16 rinv is plenty"):
                        nc.vector.tensor_copy(r16, rinv)
                    rinvs.append(r16)
                return rinvs

            def emit_fin_apply(o_ps, rinvs, i0, tag):
                """Part 2: broadcast 1/r across 64 partitions, normalize oT."""
                for h in range(2):
                    for c in range(2):
                        nc.tensor.matmul(
                            o_ps[h][64:128, c * 512 : (c + 1) * 512],
                            lhsT=ones16,
                            rhs=rinvs[h][:, c * 512 : (c + 1) * 512],
                            start=True,
                            stop=True,
                        )
                    Rs = rsp.tile([64, 1024], f32, tag="rs", name=f"Rs{tag}_{h}")
                    nc.vector.tensor_copy(Rs, o_ps[h][64:128, :])
                    nc.vector.tensor_mul(
                        oT2[h * 64 : (h + 1) * 64, i0 : i0 + 1024],
                        o_ps[h][0:64, :],
                        Rs,
                    )

            with (
                tc.tile_pool(name="scp", bufs=2, space="PSUM") as scp,
                tc.tile_pool(name="opp", bufs=2, space="PSUM") as opp,
            ):
                pending = None
                for ob in range(2):
                    for half in range(2):
                        i0 = ob * 2048 + half * 1024
                        o_ps = [
                            opp.tile([128, 1024], f32, tag="ops", name=f"o{ob}_{half}_{h}")
                            for h in range(2)
                        ]
                        pt_hist = {}

                        def emit_pv(p):
                            for h in range(2):
                                bh = ob * 2 + h
                                if PV_FP8:
                                    for c in range(2):
                                        nc.tensor.matmul(
                                            o_ps[h][0:65, c * 512 : (c + 1) * 512],
                                            lhsT=v_r[:, bh, p, 0:2, 0:65],
                                            rhs=pt_hist[p][h][:, 0:2, c * 512 : (c + 1) * 512],
                                            start=(p == 0),
                                            stop=(p == 7),
                                            perf_mode=DR,
                                        )
                                else:
                                    for b in range(2):
                                        jt = 2 * p + b
                                        for c in range(2):
                                            nc.tensor.matmul(
                                                o_ps[h][0:65, c * 512 : (c + 1) * 512],
                                                lhsT=v_r[:, bh * 16 + jt, 0:65],
                                                rhs=pt_hist[p][h][:, b, c * 512 : (c + 1) * 512],
                                                start=(jt == 0),
                                                stop=(jt == 15),
                                            )
                            del pt_hist[p]

                        for p in range(8):
                            pt_hist[p] = [
                                ptp.tile(
                                    [128, 2, 1024], pvdt, tag="pt",
                                    name=f"p{ob}_{half}_{p}_{h}",
                                )
                                for h in range(2)
                            ]
                            for b in range(2):
                                jt = 2 * p + b
                                # PV of pair p-1 rides one sub-step behind so
                                # its exp inputs are long done: no PE stall
                                if b == 1 and p >= 1:
                                    emit_pv(p - 1)
                                sc = [
                                    scp.tile(
                                        [128, 1024], f32, tag="sc",
                                        name=f"s{ob}_{half}_{jt}_{h}",
                                    )
                                    for h in range(2)
                                ]
                                # h-adjacent emission: the two K=64 matmuls
                                # target disjoint PE row-groups and overlap
                                for c in range(2):
                                    for h in range(2):
                                        nc.tensor.matmul(
                                            sc[h][:, c * 512 : (c + 1) * 512],
                                            lhsT=kT2[
                                                h * 64 : (h + 1) * 64,
                                                ob * 2048 + jt * 128 : ob * 2048 + (jt + 1) * 128,
                                            ],
                                            rhs=qT2[
                                                h * 64 : (h + 1) * 64,
                                                i0 + c * 512 : i0 + (c + 1) * 512,
                                            ],
                                            start=True,
                                            stop=True,
                                        )
                                for h in range(2):
                                    nc.scalar.activation(
                                        pt_hist[p][h][:, b, :], sc[h],
                                        AF.Exp, scale=0.125, bias=expb[:, 0:1],
                                    )
                                if DEBUG_DUMPS and ob == 0 and half == 0 and p == 0 and b == 1:
                                    ptd = big.tile([128, 2048], f32, name="ptd")
                                    nc.vector.tensor_copy(
                                        ptd, pt_hist[0][0].rearrange("p a b -> p (a b)")
                                    )
                                    nc.sync.dma_start(out=d_pt, in_=ptd)
                            # deferred finalize-apply of the previous
                            # (ob, half): its reciprocal is already done
                            if p == 0 and pending is not None:
                                emit_fin_apply(*pending)
                                pending = None
                        emit_pv(7)
                        rinvs = emit_fin_recip(
                            o_ps, f"{ob}_{half}", dbg_idx=ob * 2 + half
                        )
                        pending = (o_ps, rinvs, i0, f"{ob}_{half}")
                emit_fin_apply(*pending)
            if DEBUG_DUMPS:
                odump = big.tile([128, 4096], f32, name="odump")
                nc.vector.tensor_copy(odump, oT2)
                nc.sync.dma_start(out=d_ot2, in_=odump)

            # ---- out-projection: partial = Wo_slice.T @ oT (K=256) ----
            dma_split(wo_sb, wo)
            with tc.tile_pool(name="opj", bufs=4, space="PSUM") as pj:
                for dtb in range(8):
                    ops = [
                        pj.tile([128, 512], f32, tag="pj", name=f"pj{dtb}_{c}")
                        for c in range(4)
                    ]
                    for ob in range(2):
                        for c in range(4):
                            nc.tensor.matmul(
                                ops[c],
                                lhsT=wo_sb[:, (ob * 8 + dtb) * 128 : (ob * 8 + dtb + 1) * 128],
                                rhs=oT2[:, ob * 2048 + c * 512 : ob * 2048 + (c + 1) * 512],
                                start=(ob == 0),
                                stop=(ob == 1),
                            )
                    for c in range(4):
                        st = stp.tile([128, 512], mmdt, tag="st", name=f"st{dtb}_{c}")
                        eng = nc.vector.tensor_copy if c % 2 else nc.scalar.copy
                        eng(st, ops[c])
                        nc.sync.dma_start(
                            out=pout[
                                dtb * 128 : (dtb + 1) * 128, c * 512 : (c + 1) * 512
                            ],
                            in_=st,
                        )

    nc.compile()
    return nc


MM_DTYPE = "float16"


def _get_nc():
    key = ("nc", MM_DTYPE, PV_FP8)
    if key not in _CACHE:
        _CACHE[key] = _build_nc(MM_DTYPE)
    return _CACHE[key]


def _ensure_ntff_hook():
    """Register the NTFF profile hook module if the image lacks it."""
    import sys
    import types

    if "antenv.axon_hooks" in sys.modules:
        return
    try:
        from trn_agent_boot.trn_boot import _ntff_profile_via_ctypes
    except Exception:
        return
    hook = None
    try:
        hook = _ntff_profile_via_ctypes("/opt/axon/libaxon_pjrt.so")
    except Exception:
        hook = None
    mod = types.ModuleType("antenv.axon_hooks")
    mod._hook = hook
    mod.get_axon_ntff_profile_hook = lambda: mod._hook
    mod.set_axon_ntff_profile_hook = lambda h: setattr(mod, "_hook", h)
    sys.modules["antenv.axon_hooks"] = mod


def _make_in_maps(inputs, ext_dt):
    query = np.asarray(inputs["query"], np.float32)
    key = np.asarray(inputs["key"], np.float32)
    value = np.asarray(inputs["value"], np.float32)
    Wq = np.asarray(inputs["Wq"], np.float32)
    Wk = np.asarray(inputs["Wk"], np.float32)
    Wv = np.asarray(inputs["Wv"], np.float32)
    Wo = np.asarray(inputs["Wo"], np.float32)
    bq = np.asarray(inputs["bq"], np.float32)
    bk = np.asarray(inputs["bk"], np.float32)
    bv = np.asarray(inputs["bv"], np.float32)

    # per-batch transposed inputs [D, S]
    xT = {}
    for b in range(B):
        xT[("q", b)] = np.ascontiguousarray(query[b].T.astype(ext_dt))
        xT[("k", b)] = np.ascontiguousarray(key[b].T.astype(ext_dt))
        xT[("v", b)] = np.ascontiguousarray(value[b].T.astype(ext_dt))

    ident_np = np.zeros((128, 64), np.float32)
    ident_np[np.arange(64), np.arange(64)] = 1.0
    ident_np[64 + np.arange(64), np.arange(64)] = 1.0
    consts = {
        "c_ident": np.ascontiguousarray(ident_np.astype(ext_dt)),
        "c_ones64f": np.ones((1, 64), np.float32),
    }

    def pack_w(Wc):  # [1024, 256] -> [128, 2048] as (kk, ob) tiles
        return np.ascontiguousarray(
            Wc.reshape(8, 128, 2, 128).transpose(1, 0, 2, 3).reshape(128, 2048).astype(ext_dt)
        )

    def pack_wo(Wc):  # [256, 1024] -> [128, 2048] as (ob, dt) tiles
        return np.ascontiguousarray(
            Wc.reshape(2, 128, 8, 128).transpose(1, 0, 2, 3).reshape(128, 2048).astype(ext_dt)
        )

    in_maps = []
    for c in range(N_CORES):
        b, hs = divmod(c, 4)
        sl = slice(hs * 256, (hs + 1) * 256)
        bias6 = np.zeros((128, 6), np.float32)
        bias6[:, 0] = bq[sl][0:128]
        bias6[:, 1] = bq[sl][128:256]
        bias6[:, 2] = bk[sl][0:128]
        bias6[:, 3] = bk[sl][128:256]
        bias6[:, 4] = bv[sl][0:128]
        bias6[:, 5] = bv[sl][128:256]
        in_maps.append(
            {
                **consts,
                "xq": xT[("q", b)],
                "xk": xT[("k", b)],
                "xv": xT[("v", b)],
                "wq": pack_w(Wq[:, sl]),
                "wk": pack_w(Wk[:, sl]),
                "wv": pack_w(Wv[:, sl]),
                "wo": pack_wo(Wo[sl, :]),
                "bias6": np.ascontiguousarray(bias6),
            }
        )
    return in_maps


def _gather(results, bo):
    outT = np.zeros((B, D, S), np.float64)
    for c in range(N_CORES):
        outT[c // 4] += np.asarray(results[c]["pout"], np.float64)
    out = outT.transpose(0, 2, 1) + bo.astype(np.float64)
    return out.astype(np.float32)


def _run(inputs, trace=False):
    from concourse import bass_utils

    if trace:
        _ensure_ntff_hook()

    nc = _get_nc()
    if MM_DTYPE == "bfloat16":
        import ml_dtypes

        ext_dt = ml_dtypes.bfloat16
    elif MM_DTYPE == "float16":
        ext_dt = np.float16
    else:
        ext_dt = np.float32

    in_maps = _make_in_maps(inputs, ext_dt)
    res = bass_utils.run_bass_kernel_spmd(
        nc, in_maps, core_ids=list(range(N_CORES)), trace=trace
    )
    bo = np.asarray(inputs["bo"], np.float32)
    out = _gather(res.results, bo)
    return out.reshape(B, S, D), res


def kernel(**inputs):
    out, _ = _run(inputs, trace=False)
    return out


# revision 26
# speedup vs baseline: 1.3193x; 1.2104x over previous
"""Multi-head attention (B=2, S=2048, D=1024, H=16, Dk=64) on 8 TRN2 cores.

Sharding: batch-split x head-TP.  Core c handles batch c//4 and heads
hs*4..hs*4+3 where hs = c%4 (256 projection dims = 2 "ob" blocks of 128).

The PE clock-gate (HAM) only unthrottles for full-array matmuls, so every
attention matmul is padded to 128x128:
  - scores: per-head K tiles kpA/kpB hold the head's 64 k-dims zero-padded
    to 128 partitions (zeros annihilate the other head's q rows), so
    scoresT = kpad.T @ qT runs K=128 full-array.
  - PV: v_aug columns padded with 1.0 to M=128; PSUM rows 0:64 = o,
    rows 64:128 all = softmax row-sum (the 1-columns), which feeds
    reciprocal_approx_fast directly -- no broadcast matmul needed.
Each core:
  1. projects k/q/v = (W_slice.T @ x.T) for its 4 heads
  2. transposes vT into per-(ob,h) [j, d] blocks (cols 64:128 = 1.0)
  3. pipelined attention per (ob, half): scoresT -> exp (FD=1024 ACT)
     -> PV accumulate [128, 1024] PSUM -> 1/rowsum -> normalize into oT2
  4. partialT = Wo_slice.T @ oT  (K=256 accumulated over both obs)
Host sums 4 partials per batch, adds bo, transposes back.
All matmuls fp16 operands with fp32 PSUM accumulation.
"""

import numpy as np

D = 1024
S = 2048  # tokens per batch (= per core)
B = 2
N_CORES = 8

_CACHE = {}


def _build_nc(mm_dtype="float16"):
    import concourse.bacc as bacc
    import concourse.mybir as mybir
    import concourse.tile as tile

    dt = mybir.dt
    f32 = dt.float32
    mmdt = getattr(dt, mm_dtype)
    AF = mybir.ActivationFunctionType

    nc = bacc.Bacc("TRN2", target_bir_lowering=False, debug=False)

    xq = nc.dram_tensor("xq", [D, S], mmdt, kind="ExternalInput").ap()
    xk = nc.dram_tensor("xk", [D, S], mmdt, kind="ExternalInput").ap()
    xv = nc.dram_tensor("xv", [D, S], mmdt, kind="ExternalInput").ap()
    wq = nc.dram_tensor("wq", [128, 2048], mmdt, kind="ExternalInput").ap()
    wk = nc.dram_tensor("wk", [128, 2048], mmdt, kind="ExternalInput").ap()
    wv = nc.dram_tensor("wv", [128, 2048], mmdt, kind="ExternalInput").ap()
    wo = nc.dram_tensor("wo", [128, 2048], mmdt, kind="ExternalInput").ap()
    bias6 = nc.dram_tensor("bias6", [128, 6], f32, kind="ExternalInput").ap()
    c_ident = nc.dram_tensor("c_ident", [128, 64], mmdt, kind="ExternalInput").ap()
    c_ones64f = nc.dram_tensor("c_ones64f", [1, 64], f32, kind="ExternalInput").ap()
    pout = nc.dram_tensor("pout", [D, S], mmdt, kind="ExternalOutput").ap()

    with tile.TileContext(nc) as tc:
        from contextlib import ExitStack

        with ExitStack() as stk:
            const = stk.enter_context(tc.tile_pool(name="const", bufs=1))
            wpool = stk.enter_context(tc.tile_pool(name="w", bufs=1))
            big = stk.enter_context(tc.tile_pool(name="big", bufs=1))
            xpool = stk.enter_context(tc.tile_pool(name="xt", bufs=8))
            ptp = stk.enter_context(tc.tile_pool(name="pt", bufs=4))
            rsp = stk.enter_context(tc.tile_pool(name="rs", bufs=2))
            stp = stk.enter_context(tc.tile_pool(name="st", bufs=4))

            # ---- constants ----
            ident = const.tile([128, 64], mmdt)
            nc.sync.dma_start(out=ident, in_=c_ident)
            bias_sb = const.tile([128, 6], f32)
            nc.sync.dma_start(out=bias_sb, in_=bias6)
            # preload the exp table set while projections run
            warm = const.tile([128, 1], f32)
            nc.scalar.activation(warm, bias_sb[:, 0:1], AF.Exp, scale=0.0)

            def dma_split(dst, src, nq=4):
                """Split a [128, N] HBM->SBUF load across nq DMA queues
                (per-queue BW is ~23 GB/s; a 512KB tile on one queue = 23us)."""
                step = 128 // nq
                for q in range(nq):
                    nc.sync.dma_start(
                        out=dst[q * step : (q + 1) * step, :],
                        in_=src[q * step : (q + 1) * step, :],
                    )

            # ---- weights (wo deferred until the out-projection) ----
            wq_sb = wpool.tile([128, 2048], mmdt)
            wk_sb = wpool.tile([128, 2048], mmdt)
            wv_sb = wpool.tile([128, 2048], mmdt)
            wo_sb = wpool.tile([128, 2048], mmdt)
            dma_split(wk_sb, wk)
            dma_split(wq_sb, wq)
            dma_split(wv_sb, wv)

            # ---- persistent activations ----
            qT2 = big.tile([128, 4096], mmdt)  # [dh within ob, ob*2048 + tok]
            vT2 = big.tile([128, 4096], mmdt)
            oT2 = big.tile([128, 4096], mmdt)
            # per-head zero-padded K: kpads[h] holds head h's k rows in
            # partitions h*64:(h+1)*64, zeros elsewhere
            kpA = big.tile([128, 4096], mmdt)
            kpB = big.tile([128, 4096], mmdt)
            nc.vector.memset(kpA, 0.0)
            nc.vector.memset(kpB, 0.0)
            kpads = [kpA, kpB]
            # v_aug blocks [j, 128]: cols 0:64 = V block, cols 64:128 = 1.0
            v_sb = big.tile([128, 4 * 16 * 128], mmdt)
            nc.vector.memset(v_sb, 1.0)
            v_r = v_sb.rearrange("p (t c) -> p t c", c=128)

            def emit_proj(x_dram, w_sb, pnm, pp, writeback):
                """acc[ob] = W[:, ob].T @ x for both ob blocks; `writeback(ob,
                n, acc)` copies psum->SBUF.  Per-ob acc quads rotate through
                the shared 8-slot pool so projections pipeline stall-free."""
                x_ts = []
                for kk in range(8):
                    x_t = xpool.tile([128, 2048], mmdt, tag="xt", name=f"x{pnm}{kk}")
                    dma_split(x_t, x_dram[kk * 128 : (kk + 1) * 128, :])
                    x_ts.append(x_t)
                for ob in range(2):
                    acc = [
                        pp.tile([128, 512], f32, tag="pp", name=f"acc{pnm}{ob}_{n}")
                        for n in range(4)
                    ]
                    for kk in range(8):
                        for n in range(4):
                            nc.tensor.matmul(
                                acc[n],
                                lhsT=w_sb[:, (kk * 2 + ob) * 128 : (kk * 2 + ob + 1) * 128],
                                rhs=x_ts[kk][:, n * 512 : (n + 1) * 512],
                                start=(kk == 0),
                                stop=(kk == 7),
                            )
                    for n in range(4):
                        writeback(ob, n, acc[n])

            def wb_simple(dst, bias_col0):
                def wb(ob, n, acc):
                    dstv = dst[:, ob * 2048 + n * 512 : ob * 2048 + (n + 1) * 512]
                    bv = bias_sb[:, bias_col0 + ob : bias_col0 + ob + 1]
                    if n < 2:
                        nc.vector.tensor_scalar_add(dstv, acc, bv)
                    else:
                        nc.scalar.activation(dstv, acc, AF.Identity, bias=bv)
                return wb

            def wb_kpad(ob, n, acc):
                """k rows split into the per-head zero-padded tiles."""
                cs = slice(ob * 2048 + n * 512, ob * 2048 + (n + 1) * 512)
                for h in range(2):
                    rs_ = slice(h * 64, (h + 1) * 64)
                    bv = bias_sb[rs_, 2 + ob : 3 + ob]
                    if h == 0:
                        nc.vector.tensor_scalar_add(kpads[0][rs_, cs], acc[rs_, :], bv)
                    else:
                        nc.scalar.activation(
                            kpads[1][rs_, cs], acc[rs_, :], AF.Identity, bias=bv
                        )

            def emit_transp():
                """vT2 -> v_sb [j, d] blocks (cols 64:128 stay 1.0)."""
                with tc.tile_pool(name="tp", bufs=3, space="PSUM") as tpp:
                    for ob in range(2):
                        for h in range(2):
                            bh = ob * 2 + h
                            for g in range(4):
                                tp = tpp.tile(
                                    [128, 4 * 64], mmdt, tag="tp", name=f"tp{bh}_{g}"
                                )
                                for u in range(4):
                                    jb = g * 4 + u
                                    nc.tensor.transpose(
                                        tp[:, u * 64 : (u + 1) * 64],
                                        vT2[
                                            h * 64 : (h + 1) * 64,
                                            ob * 2048 + jb * 128 : ob * 2048 + (jb + 1) * 128,
                                        ],
                                        ident[h * 64 : (h + 1) * 64, :],
                                    )
                                tp_r = tp.rearrange("p (t c) -> p t c", c=64)
                                nc.scalar.copy(
                                    v_r[:, bh * 16 + g * 4 : bh * 16 + g * 4 + 4, 0:64],
                                    tp_r,
                                )

            # =========== emission schedule ===========
            with tc.tile_pool(name="pp", bufs=8, space="PSUM") as pp:
                emit_proj(xk, wk_sb, "k", pp, wb_kpad)
                emit_proj(xq, wq_sb, "q", pp, wb_simple(qT2, 0))
                emit_proj(xv, wv_sb, "v", pp, wb_simple(vT2, 4))
            emit_transp()

            # ---- attention: pipelined over (ob, half, jt-pair) ----
            def emit_fin_stage(o_ps, tag):
                """Rows 64:128 all hold the row-sum (1.0-padded v columns):
                stage to SBUF (custom-DVE ops ignore input base partition),
                then fast-reciprocal."""
                Rss = []
                for h in range(2):
                    rsum = rsp.tile([64, 1024], f32, tag="rsm", name=f"rm{tag}_{h}")
                    nc.vector.tensor_copy(rsum, o_ps[h][64:128, :])
                    Rs = rsp.tile([64, 1024], f32, tag="rs", name=f"Rs{tag}_{h}")
                    nc.vector.reciprocal_approx_fast(Rs, rsum)
                    Rss.append(Rs)
                return Rss

            def emit_fin_mul(o_ps, Rss, i0, tag):
                for h in range(2):
                    nc.vector.tensor_mul(
                        oT2[h * 64 : (h + 1) * 64, i0 : i0 + 1024],
                        o_ps[h][0:64, :],
                        Rss[h],
                    )

            with (
                tc.tile_pool(name="scp", bufs=2, space="PSUM") as scp,
                tc.tile_pool(name="opp", bufs=2, space="PSUM") as opp,
            ):
                pending = None
                for ob in range(2):
                    for half in range(2):
                        i0 = ob * 2048 + half * 1024
                        o_ps = [
                            opp.tile([128, 1024], f32, tag="ops", name=f"o{ob}_{half}_{h}")
                            for h in range(2)
                        ]
                        pt_hist = {}

                        def emit_pv(p):
                            for h in range(2):
                                bh = ob * 2 + h
                                for b in range(2):
                                    jt = 2 * p + b
                                    for c in range(2):
                                        nc.tensor.matmul(
                                            o_ps[h][:, c * 512 : (c + 1) * 512],
                                            lhsT=v_r[:, bh * 16 + jt, :],
                                            rhs=pt_hist[p][h][:, b, c * 512 : (c + 1) * 512],
                                            start=(jt == 0),
                                            stop=(jt == 15),
                                        )
                            del pt_hist[p]

                        for p in range(8):
                            pt_hist[p] = [
                                ptp.tile(
                                    [128, 2, 1024], mmdt, tag="pt",
                                    name=f"p{ob}_{half}_{p}_{h}",
                                )
                                for h in range(2)
                            ]
                            for b in range(2):
                                jt = 2 * p + b
                                # PV of pair p-1 rides one sub-step behind so
                                # its exp inputs are long done: no PE stall
                                if b == 1 and p >= 1:
                                    emit_pv(p - 1)
                                sc = [
                                    scp.tile(
                                        [128, 1024], f32, tag="sc",
                                        name=f"s{ob}_{half}_{jt}_{h}",
                                    )
                                    for h in range(2)
                                ]
                                for c in range(2):
                                    for h in range(2):
                                        nc.tensor.matmul(
                                            sc[h][:, c * 512 : (c + 1) * 512],
                                            lhsT=kpads[h][
                                                :,
                                                ob * 2048 + jt * 128 : ob * 2048 + (jt + 1) * 128,
                                            ],
                                            rhs=qT2[:, i0 + c * 512 : i0 + (c + 1) * 512],
                                            start=True,
                                            stop=True,
                                        )
                                for h in range(2):
                                    nc.scalar.activation(
                                        pt_hist[p][h][:, b, :], sc[h], AF.Exp, scale=0.125
                                    )
                            # deferred normalize of the previous (ob, half):
                            # its reciprocal is already done
                            if p == 0 and pending is not None:
                                emit_fin_mul(*pending)
                                pending = None
                        emit_pv(7)
                        Rss = emit_fin_stage(o_ps, f"{ob}_{half}")
                        pending = (o_ps, Rss, i0, f"{ob}_{half}")
                emit_fin_mul(*pending)

            # ---- out-projection: partial = Wo_slice.T @ oT (K=256) ----
            dma_split(wo_sb, wo)
            with tc.tile_pool(name="opj", bufs=4, space="PSUM") as pj:
                for dtb in range(8):
                    ops = [
                        pj.tile([128, 512], f32, tag="pj", name=f"pj{dtb}_{c}")
                        for c in range(4)
                    ]
                    for ob in range(2):
                        for c in range(4):
                            nc.tensor.matmul(
                                ops[c],
                                lhsT=wo_sb[:, (ob * 8 + dtb) * 128 : (ob * 8 + dtb + 1) * 128],
                                rhs=oT2[:, ob * 2048 + c * 512 : ob * 2048 + (c + 1) * 512],
                                start=(ob == 0),
                                stop=(ob == 1),
                            )
                    for c in range(4):
                        st = stp.tile([128, 512], mmdt, tag="st", name=f"st{dtb}_{c}")
                        eng = nc.vector.tensor_copy if c % 2 else nc.scalar.copy
                        eng(st, ops[c])
                        nc.sync.dma_start(
                            out=pout[
                                dtb * 128 : (dtb + 1) * 128, c * 512 : (c + 1) * 512
                            ],
                            in_=st,
                        )

    nc.compile()
    return nc


MM_DTYPE = "float16"


def _get_nc():
    key = ("nc", MM_DTYPE)
    if key not in _CACHE:
        _CACHE[key] = _build_nc(MM_DTYPE)
    return _CACHE[key]


def _ensure_ntff_hook():
    """Register the NTFF profile hook module if the image lacks it."""
    import sys
    import types

    if "antenv.axon_hooks" in sys.modules:
        return
    try:
        from trn_agent_boot.trn_boot import _ntff_profile_via_ctypes
    except Exception:
        return
    hook = None
    try:
        hook = _ntff_profile_via_ctypes("/opt/axon/libaxon_pjrt.so")
    except Exception:
        hook = None
    mod = types.ModuleType("antenv.axon_hooks")
    mod._hook = hook
    mod.get_axon_ntff_profile_hook = lambda: mod._hook
    mod.set_axon_ntff_profile_hook = lambda h: setattr(mod, "_hook", h)
    sys.modules["antenv.axon_hooks"] = mod


def _make_in_maps(inputs, ext_dt):
    query = np.asarray(inputs["query"], np.float32)
    key = np.asarray(inputs["key"], np.float32)
    value = np.asarray(inputs["value"], np.float32)
    Wq = np.asarray(inputs["Wq"], np.float32)
    Wk = np.asarray(inputs["Wk"], np.float32)
    Wv = np.asarray(inputs["Wv"], np.float32)
    Wo = np.asarray(inputs["Wo"], np.float32)
    bq = np.asarray(inputs["bq"], np.float32)
    bk = np.asarray(inputs["bk"], np.float32)
    bv = np.asarray(inputs["bv"], np.float32)

    # per-batch transposed inputs [D, S]
    xT = {}
    for b in range(B):
        xT[("q", b)] = np.ascontiguousarray(query[b].T.astype(ext_dt))
        xT[("k", b)] = np.ascontiguousarray(key[b].T.astype(ext_dt))
        xT[("v", b)] = np.ascontiguousarray(value[b].T.astype(ext_dt))

    ident_np = np.zeros((128, 64), np.float32)
    ident_np[np.arange(64), np.arange(64)] = 1.0
    ident_np[64 + np.arange(64), np.arange(64)] = 1.0
    consts = {
        "c_ident": np.ascontiguousarray(ident_np.astype(ext_dt)),
        "c_ones64f": np.ones((1, 64), np.float32),
    }

    def pack_w(Wc):  # [1024, 256] -> [128, 2048] as (kk, ob) tiles
        return np.ascontiguousarray(
            Wc.reshape(8, 128, 2, 128).transpose(1, 0, 2, 3).reshape(128, 2048).astype(ext_dt)
        )

    def pack_wo(Wc):  # [256, 1024] -> [128, 2048] as (ob, dt) tiles
        return np.ascontiguousarray(
            Wc.reshape(2, 128, 8, 128).transpose(1, 0, 2, 3).reshape(128, 2048).astype(ext_dt)
        )

    in_maps = []
    for c in range(N_CORES):
        b, hs = divmod(c, 4)
        sl = slice(hs * 256, (hs + 1) * 256)
        bias6 = np.zeros((128, 6), np.float32)
        bias6[:, 0] = bq[sl][0:128]
        bias6[:, 1] = bq[sl][128:256]
        bias6[:, 2] = bk[sl][0:128]
        bias6[:, 3] = bk[sl][128:256]
        bias6[:, 4] = bv[sl][0:128]
        bias6[:, 5] = bv[sl][128:256]
        in_maps.append(
            {
                **consts,
                "xq": xT[("q", b)],
                "xk": xT[("k", b)],
                "xv": xT[("v", b)],
                "wq": pack_w(Wq[:, sl]),
                "wk": pack_w(Wk[:, sl]),
                "wv": pack_w(Wv[:, sl]),
                "wo": pack_wo(Wo[sl, :]),
                "bias6": np.ascontiguousarray(bias6),
            }
        )
    return in_maps


def _gather(results, bo):
    outT = np.zeros((B, D, S), np.float64)
    for c in range(N_CORES):
        outT[c // 4] += np.asarray(results[c]["pout"], np.float64)
    out = outT.transpose(0, 2, 1) + bo.astype(np.float64)
    return out.astype(np.float32)


def _run(inputs, trace=False):
    from concourse import bass_utils

    if trace:
        _ensure_ntff_hook()

    nc = _get_nc()
    if MM_DTYPE == "bfloat16":
        import ml_dtypes

        ext_dt = ml_dtypes.bfloat16
    elif MM_DTYPE == "float16":
        ext_dt = np.float16
    else:
        ext_dt = np.float32

    in_maps = _make_in_maps(inputs, ext_dt)
    res = bass_utils.run_bass_kernel_spmd(
        nc, in_maps, core_ids=list(range(N_CORES)), trace=trace
    )
    bo = np.asarray(inputs["bo"], np.float32)
    out = _gather(res.results, bo)
    return out.reshape(B, S, D), res


def kernel(**inputs):
    out, _ = _run(inputs, trace=False)
    return out
